# revision 4
# baseline (speedup 1.0000x reference)
"""DeMash kernel for Trainium2 (8 NeuronCores, Bass/Tile).

Math: Y = X @ C^H over rows n = (B,T,S) flattened, with a subcarrier
gather before and scatter after. Split into real arithmetic:
    Yr = Xr @ Cr^T + Xi @ Ci^T
    Yi = Xi @ Cr^T - Xr @ Ci^T
Sharding: data-parallel over batch (axis 0), 32 batches -> 256 rows per
core; C replicated. On-device: fp32r matmuls (full PE rate, ~1e-4 rel
err), stationary = X^T tiles, moving = C^T slabs, PSUM accumulate over
the L=1512 contraction in 12 tiles of 126.
"""

import numpy as np
import concourse.bass as bass
import concourse.mybir as mybir
from concourse import bacc
from concourse.tile import TileContext
from concourse.bass_utils import run_bass_kernel_spmd

B, T, S, SYM, FFT = 256, 4, 2, 14, 128
NSC = 108
L = SYM * NSC                   # 1512
NCORES = 8
ROWS = (B // NCORES) * T * S    # 256 rows per core
NT = ROWS // 128                # 2 row tiles of 128
KT, KP = 12, 126                # contraction tiles: 12 x 126 = 1512
NCH, NW = 3, 504                # output column chunks: 3 x 504 = 1512

F32 = mybir.dt.float32
F32R = mybir.dt.float32r

TRACE = False                   # test harness flips this for profiling
LAST_RESULTS = None             # stashed BassKernelResults for the harness

_NC = None


def _build_program():
    nc = bacc.Bacc("TRN2", target_bir_lowering=False, debug=False)
    XX = nc.dram_tensor("XX", [KT, KP, 2 * ROWS], F32R, kind="ExternalInput")
    CC = nc.dram_tensor("CC", [KT, NCH, KP, 2 * NW], F32R, kind="ExternalInput")
    YR = nc.dram_tensor("YR", [ROWS, L], F32, kind="ExternalOutput")
    YI = nc.dram_tensor("YI", [ROWS, L], F32, kind="ExternalOutput")

    with TileContext(nc) as tc:
        with (
            tc.tile_pool(name="xp", bufs=1) as xp,
            tc.tile_pool(name="cp", bufs=2 * KT) as cp,
            tc.tile_pool(name="op", bufs=4) as op,
            tc.tile_pool(name="pp", bufs=2, space="PSUM") as pp,
        ):
            # Resident stationary operand: X^T, laid out [126, (k, ri, n)]
            xt = xp.tile([128, KT * 2 * ROWS], F32R, tag="xt")
            for k in range(KT):
                nc.sync.dma_start(
                    out=xt[:KP, k * 2 * ROWS:(k + 1) * 2 * ROWS], in_=XX[k]
                )
            # -Xr^T for the imaginary accumulation (Yi = Xi@Cr^T + (-Xr)@Ci^T)
            xn = xp.tile([128, KT * ROWS], F32R, tag="xn")
            for k in range(KT):
                nc.vector.tensor_scalar_mul(
                    xn[:KP, k * ROWS:(k + 1) * ROWS],
                    xt[:KP, k * 2 * ROWS:k * 2 * ROWS + ROWS],
                    -1.0,
                )

            def xsl(k, ri, n):
                off = k * 2 * ROWS + ri * ROWS + n * 128
                return xt[:KP, off:off + 128]

            def xnsl(k, n):
                off = k * ROWS + n * 128
                return xn[:KP, off:off + 128]

            for mc in range(NCH):
                cts = []
                for k in range(KT):
                    ct = cp.tile([128, 2 * NW], F32R, tag="cc")
                    nc.sync.dma_start(out=ct[:KP, :], in_=CC[k, mc])
                    cts.append(ct)
                for n in range(NT):
                    pr = pp.tile([128, NW], F32, tag="pr")
                    pi = pp.tile([128, NW], F32, tag="pi")
                    for k in range(KT):
                        cr = cts[k][:KP, 0:NW]
                        ci = cts[k][:KP, NW:2 * NW]
                        nc.tensor.matmul(pr[:], xsl(k, 0, n), cr,
                                         start=(k == 0), stop=False)
                        nc.tensor.matmul(pr[:], xsl(k, 1, n), ci,
                                         start=False, stop=(k == KT - 1))
                        nc.tensor.matmul(pi[:], xsl(k, 1, n), cr,
                                         start=(k == 0), stop=False)
                        nc.tensor.matmul(pi[:], xnsl(k, n), ci,
                                         start=False, stop=(k == KT - 1))
                    yr = op.tile([128, NW], F32, tag="yr")
                    yi = op.tile([128, NW], F32, tag="yi")
                    nc.vector.tensor_copy(out=yr[:], in_=pr[:])
                    nc.vector.tensor_copy(out=yi[:], in_=pi[:])
                    rsl = slice(n * 128, (n + 1) * 128)
                    csl = slice(mc * NW, (mc + 1) * NW)
                    nc.sync.dma_start(out=YR[rsl, csl], in_=yr[:])
                    nc.sync.dma_start(out=YI[rsl, csl], in_=yi[:])
    nc.compile()
    return nc


def _get_nc():
    global _NC
    if _NC is None:
        _NC = _build_program()
    return _NC


def kernel(x_real, x_imag, C_real, C_imag, sc_ind):
    global LAST_RESULTS
    xr = np.asarray(x_real, dtype=np.float32)
    xi = np.asarray(x_imag, dtype=np.float32)
    cr = np.asarray(C_real, dtype=np.float32)
    ci = np.asarray(C_imag, dtype=np.float32)
    sc = np.asarray(sc_ind)

    # Host prep: gather effective subcarriers, flatten, transpose.
    idx = sc.astype(np.int64)
    xgr = xr[..., idx].reshape(B * T * S, L)      # [2048, 1512]
    xgi = xi[..., idx].reshape(B * T * S, L)
    xrT = np.ascontiguousarray(xgr.T)             # [1512, 2048]
    xiT = np.ascontiguousarray(xgi.T)

    # C^T slabs, r/i concatenated: CC[k, mc, p, 0:504]=Cr^T, [504:]=Ci^T
    crT = cr.T.reshape(KT, KP, NCH, NW)
    ciT = ci.T.reshape(KT, KP, NCH, NW)
    CC = np.empty((KT, NCH, KP, 2 * NW), dtype=np.float32)
    CC[..., 0:NW] = crT.transpose(0, 2, 1, 3)
    CC[..., NW:] = ciT.transpose(0, 2, 1, 3)
    CC = np.ascontiguousarray(CC)

    in_maps = []
    for c in range(NCORES):
        cols = slice(c * ROWS, (c + 1) * ROWS)
        XXc = np.empty((KT, KP, 2 * ROWS), dtype=np.float32)
        XXc[..., 0:ROWS] = xrT[:, cols].reshape(KT, KP, ROWS)
        XXc[..., ROWS:] = xiT[:, cols].reshape(KT, KP, ROWS)
        in_maps.append({"XX": np.ascontiguousarray(XXc), "CC": CC})

    nc = _get_nc()
    res = run_bass_kernel_spmd(
        nc, in_maps, core_ids=list(range(NCORES)), trace=TRACE
    )
    LAST_RESULTS = res

    yr_full = np.concatenate([r["YR"] for r in res.results], axis=0)
    yi_full = np.concatenate([r["YI"] for r in res.results], axis=0)

    out = np.zeros((2, B, T, S, SYM, FFT), dtype=np.float32)
    out[0].reshape(B * T * S, SYM, FFT)[:, :, idx] = yr_full.reshape(
        B * T * S, SYM, NSC
    )
    out[1].reshape(B * T * S, SYM, FFT)[:, :, idx] = yi_full.reshape(
        B * T * S, SYM, NSC
    )
    return out
